# revision 35
# baseline (speedup 1.0000x reference)
"""MultiHeadSemGConv Trainium2 kernel.

Computes, for x:[B,N,CIN], W:[H,2,CIN,HC], e:[H,N*K], bias:[H,HC],
rows/cols:[N*K] (int32 edge list):

    h = einsum('bnc,hscd->shbnd', x, W)             # two projections per head
    A = softmax(scatter(e at (rows,cols), NEG))     # [H,N,N]
    out[h,b] = diag(A)*h0 + (A - diag)@h1 + bias    # -> [B,N,H*HC]

Strategy: pure data-parallel over batch across 8 NeuronCores.  The tiny
[H,98,98] adjacency softmax is precomputed on host; the heavy lifting
(x projection + graph mixing over 100MB of activations) runs on device:

  per core (128 samples, x pre-cast to fp16 host-side):
    - DMA x16 in flat 128-partition tiles (group 0: per-pair HWDGE loads
      on both queues for low head latency; later groups: bulk SWDGE)
    - PE transpose (matmul with identity) -> xT chunk tiles
      [c(2x128), 16*98+30 cols] fp16 in SBUF (30-col overlap keeps every
      per-sample phase-1 stationary at m=128)
    - phase 1, per sample b: h[128,512] = xT[:, 98b:98b+128].T @ Wall
      (2 accumulating fp16 matmuls, f32 PSUM), 2 samples per PSUM tile;
      rows [:98] copied per-sample-half on scalar+vector in parallel into
      one of two persistent h tiles whose row 98 holds the bias pattern
    - phase 2, per 8-sample group, per head: ONE matmul with the
      host-built A_off^T (contract k=99: 98 nodes + bias row), then a
      fused DVE op  out = dg (.) h0 + psum  adds the diagonal part.
      Phase 2 of group g is interleaved into phase 1 of group g+1.
    - DMA out f32
"""

import os
import sys

import numpy as np

try:
    import concourse.bass as bass  # noqa: F401
except Exception:  # pragma: no cover - fresh grading dir fallback
    for p in ("/opt/trn_rl_repo", "/root/.axon_site/_ro/trn_rl_repo"):
        if os.path.isdir(p) and p not in sys.path:
            sys.path.insert(0, p)
    import concourse.bass as bass  # noqa: F401

# ---------------------------------------------------------------- constants
NLM = 98          # landmarks (graph nodes)
HEADS = 4
CIN = 256
HC = 64
HD = 512          # h width = 2 (s) * 4 (heads) * 64 (d)
B = 1024
NCORES = 8
NS = B // NCORES  # samples per core = 128
P = 128
G = 8             # samples per output group
NGRP = NS // G    # 16 groups per core
OVL = 30          # overlap cols so every phase-1 lhsT can be m=128
NEG = -9e15

CHS = 16                    # samples per xT chunk
NCH = NS // CHS             # 8 chunks
CHW = CHS * NLM             # 1568 cols per chunk (+OVL)
NFT = NS * NLM // P         # 98 flat 128-row tiles
NPAIR = NFT // 2            # 49 transpose pairs
DGF = 14                    # flat tiles per input DMA group
NDG = NFT // DGF            # 7 DMA groups
SPL = 7                     # pairs in the split (head) group 0

_CACHE = {}


def _build_nc():
    import concourse.mybir as mybir
    import concourse.tile as tile
    from concourse import bacc

    f16 = mybir.dt.float16
    f32 = mybir.dt.float32
    MUL = mybir.AluOpType.mult
    ADD = mybir.AluOpType.add

    nc = bacc.Bacc(None, target_bir_lowering=False)

    x16 = nc.dram_tensor("x16", [NS * NLM, CIN], f16, kind="ExternalInput")
    wall = nc.dram_tensor("wall", [P, 2, HD], f16, kind="ExternalInput")
    gmat = nc.dram_tensor("gmat", [P, HEADS * P], f16, kind="ExternalInput")
    dgvt = nc.dram_tensor("dgvt", [NLM, HEADS], f32, kind="ExternalInput")
    brow = nc.dram_tensor("brow", [1, G * HD], f16, kind="ExternalInput")
    ident = nc.dram_tensor("ident", [P, P], f16, kind="ExternalInput")
    out = nc.dram_tensor("out", [NS * NLM, CIN], f32, kind="ExternalOutput")

    with tile.TileContext(nc) as tc:
        with (
            tc.tile_pool(name="const", bufs=1) as constp,
            tc.tile_pool(name="x0", bufs=1) as x0p,
            tc.tile_pool(name="xin", bufs=3) as xinp,
            tc.tile_pool(name="xt", bufs=1) as xtp,
            tc.tile_pool(name="hg", bufs=1) as hgp,
            tc.tile_pool(name="osb", bufs=4) as osbp,
            tc.tile_pool(name="ptr", bufs=2, space="PSUM") as ptrp,
            tc.tile_pool(name="phs", bufs=2, space="PSUM") as phsp,
            tc.tile_pool(name="pout", bufs=2, space="PSUM") as poutp,
        ):
            dgv_sb = constp.tile([NLM, HEADS], f32, tag="dgv")
            ident_sb = constp.tile([P, P], f16, tag="ident")
            nc.sync.dma_start(ident_sb[:], ident[:])

            xs16 = [
                x0p.tile([P, 2, CIN], f16, tag=f"xs{pr}", name=f"xs{pr}")
                for pr in range(SPL)
            ]
            wall_sb = constp.tile([P, 2, HD], f16, tag="wall")
            gm_sb = constp.tile([P, HEADS * P], f16, tag="gmat")

            def pair_dma(pr):
                # full-pair DMA; alternate the two HWDGE queues so
                # descriptor-gen parallelizes
                b0 = pr * 2 * P
                src = x16[b0 : b0 + 2 * P, :].rearrange("(t p) c -> p t c", p=P)
                eng = nc.sync if pr % 2 == 0 else nc.scalar
                eng.dma_start(xs16[pr][:], src)

            pair_dma(0)
            nc.scalar.dma_start(wall_sb[:], wall[:])
            for pr in range(1, 4):
                pair_dma(pr)

            hgt = [
                hgp.tile([P, G * HD], f16, tag=f"hg{i}", name=f"hg{i}")
                for i in range(2)
            ]

            xt = [
                xtp.tile([P, 2, CHW + OVL], f16, tag=f"xt{k}", name=f"xt{k}")
                for k in range(NCH)
            ]
            nc.vector.memset(xt[NCH - 1][:, :, CHW:], 0.0)

            def route_piece(g0, ptr, off, w):
                """Copy ptr[:, :, off:off+w] (global xT cols [g0,g0+w)) into
                the chunk tiles, including overlap duplication."""
                while w > 0:
                    k = g0 // CHW
                    lo = g0 - k * CHW
                    pw = min(w, CHW - lo)
                    nc.scalar.copy(
                        out=xt[k][:, :, lo : lo + pw],
                        in_=ptr[:, :, off : off + pw],
                    )
                    # overlap region of the previous chunk
                    if k > 0 and lo < OVL:
                        ow = min(pw, OVL - lo)
                        nc.scalar.copy(
                            out=xt[k - 1][:, :, CHW + lo : CHW + lo + ow],
                            in_=ptr[:, :, off : off + ow],
                        )
                    g0 += pw
                    off += pw
                    w -= pw

            def transpose_pair(xin_ap, gft):
                """Transpose 2 flat tiles (xin_ap: [P, 2, CIN] fp16) whose
                first global flat-tile index is gft."""
                ptr = ptrp.tile([P, 2, 2 * P], f32, tag="ptr")
                for a in range(2):
                    for cc in range(2):
                        nc.tensor.matmul(
                            ptr[:, cc, a * P : (a + 1) * P],
                            xin_ap[:, a, cc * P : (cc + 1) * P],
                            ident_sb[:],
                            start=True,
                            stop=True,
                        )
                route_piece(gft * P, ptr, 0, 2 * P)

            # ---- bulk input: SWDGE, DGF flat tiles a shot ---------------
            # The SWDGE descriptor-gens would otherwise all fire at t=0 and
            # flood the DMA rings ahead of the latency-critical head loads.
            # The Tile scheduler reorders by data deps, so gens 1-2 are
            # gated with a WAW dep: a tiny copy (itself gated on a head
            # pair load) into the DMA's own destination tile.  Later gens
            # are paced by the xin pool WAR (bufs=3).
            xin_tiles = {}

            def emit_a_dma(dg):
                xin = xinp.tile([P, DGF, CIN], f16, tag="xin")
                if dg == 1:
                    nc.gpsimd.tensor_copy(xin[0:1, 0, 0:4], xs16[1][0:1, 0, 0:4])
                elif dg == 2:
                    nc.gpsimd.tensor_copy(xin[0:1, 0, 0:4], xs16[3][0:1, 0, 0:4])
                base = dg * DGF * P
                nc.gpsimd.dma_start(
                    xin[:],
                    x16[base : base + DGF * P, :].rearrange(
                        "(t p) c -> p t c", p=P
                    ),
                )
                xin_tiles[dg] = xin

            def emit_pair_T(tp):
                """Transpose bulk pair tp (pairs SPL.. come from bulk)."""
                ft = tp * 2
                dg, pr = divmod(ft, DGF)
                xin = xin_tiles[dg]
                transpose_pair(xin[:, pr : pr + 2, :], ft)

            hg3s = [h[:].rearrange("p (s f) -> p s f", s=G) for h in hgt]

            def emit_p2_head(gi, hd, osb3):
                """Phase 2 for one head of group gi: one k=99 matmul
                (A_off^T + bias row), then fused  out = dg (.) h0 + psum."""
                hg3 = hg3s[gi % 2]
                pouts = poutp.tile([P, G * HC], f32, tag="pout")
                po3 = pouts[:].rearrange("p (s f) -> p s f", s=G)
                nc.tensor.matmul(
                    po3,
                    gm_sb[0:99, hd * P : (hd + 1) * P],
                    hg3[0:99, :, 256 + hd * HC : 256 + (hd + 1) * HC],
                    start=True,
                    stop=True,
                )
                nc.vector.scalar_tensor_tensor(
                    out=osb3[:, :, hd * HC : (hd + 1) * HC],
                    in0=hg3[0:98, :, hd * HC : (hd + 1) * HC],
                    scalar=dgv_sb[:, hd : hd + 1],
                    in1=po3[0:98],
                    op0=MUL,
                    op1=ADD,
                )

            def store(gi, osb3, s0=0, s1=G):
                ov = out[gi * G * NLM : (gi + 1) * G * NLM, :].rearrange(
                    "(s i) c -> i s c", s=G
                )
                nc.sync.dma_start(ov[:, s0:s1], osb3[:, s0:s1])

            osb_t = {}

            def emit_b_phase1(gi, prev):
                """Phase 1 for G samples of gi; phase 2 of group `prev`
                interleaved between the pairs."""
                hgrp = hgt[gi % 2]
                if prev is not None:
                    osb = osbp.tile([NLM, G * 256], f32, tag="osb")
                    osb3 = osb[:].rearrange("p (s c) -> p s c", s=G)
                    osb_t[prev] = osb3
                ck = gi // 2
                for pi in range(G // 2):
                    hps = phsp.tile([P, 2, HD], f32, tag="hps")
                    for a in range(2):
                        b = gi * G + pi * 2 + a
                        lb = b - ck * CHS
                        for cc in range(2):
                            nc.tensor.matmul(
                                hps[:, a, :],
                                xt[ck][:, cc, NLM * lb : NLM * lb + P],
                                wall_sb[:, cc, :],
                                start=(cc == 0),
                                stop=(cc == 1),
                            )
                    # copy each sample half separately so the two engines
                    # drain a pair in parallel — a serial 1.1us copy stalls
                    # the pair+2 matmuls via the phs pool WAR
                    for a in range(2):
                        dsta = hgrp[
                            0:98, (pi * 2 + a) * HD : (pi * 2 + a + 1) * HD
                        ]
                        if a == 0:
                            nc.vector.tensor_copy(dsta, hps[0:98, a, :])
                        else:
                            nc.scalar.copy(out=dsta, in_=hps[0:98, a, :])
                    if prev is not None:
                        if pi < 3:
                            emit_p2_head(prev, pi, osb_t[prev])
                        else:
                            emit_p2_head(prev, 3, osb_t[prev])
                            store(prev, osb_t[prev])

            def emit_p2_flush(gi):
                """Phase 2 for the final group, split in half-groups with
                split stores for a short kernel tail."""
                osb = osbp.tile([NLM, G * 256], f32, tag="osb")
                osb3 = osb[:].rearrange("p (s c) -> p s c", s=G)
                hg3 = hg3s[gi % 2]
                for half in range(2):
                    s0, s1 = half * 4, half * 4 + 4
                    for hd in range(HEADS):
                        pouts = poutp.tile([P, G * HC], f32, tag="pout")
                        po3 = pouts[:].rearrange("p (s f) -> p s f", s=G)
                        nc.tensor.matmul(
                            po3[:, s0:s1, :],
                            gm_sb[0:99, hd * P : (hd + 1) * P],
                            hg3[0:99, s0:s1, 256 + hd * HC : 256 + (hd + 1) * HC],
                            start=True,
                            stop=True,
                        )
                        nc.vector.scalar_tensor_tensor(
                            out=osb3[:, s0:s1, hd * HC : (hd + 1) * HC],
                            in0=hg3[0:98, s0:s1, hd * HC : (hd + 1) * HC],
                            scalar=dgv_sb[:, hd : hd + 1],
                            in1=po3[0:98, s0:s1],
                            op0=MUL,
                            op1=ADD,
                        )
                    if half == 0:
                        store(gi, osb3, 0, 4)
                    else:
                        store(gi, osb3, 4, 6)
                        store(gi, osb3, 6, 8)

            # ---- main emission ------------------------------------------
            t_need = [
                -(-(CHW * (k + 1) + OVL) // (2 * P)) for k in range(NCH)
            ]
            t_need[NCH - 1] = NPAIR
            dg_need = [min(-(-(2 * t_need[k]) // DGF), NDG) for k in range(NCH)]

            # group-0 head: 4 pairs -> b0 -> 3 pairs
            transpose_pair(xs16[0][:], 0)
            transpose_pair(xs16[1][:], 2)
            for pr in range(4, SPL):
                pair_dma(pr)
            nc.sync.dma_start(gm_sb[:], gmat[:])
            nc.scalar.dma_start(dgv_sb[:], dgvt[:])
            nc.sync.dma_start(hgt[0][98:99, :], brow[:])
            nc.sync.dma_start(hgt[1][98:99, :], brow[:])
            transpose_pair(xs16[2][:], 4)
            transpose_pair(xs16[3][:], 6)
            dma_groups = 1  # group 0 done via split path
            t_pairs = SPL
            prev = None
            for gi in range(NGRP):
                ck = gi // 2
                while dma_groups < dg_need[min((gi + 2) // 2, NCH - 1)]:
                    emit_a_dma(dma_groups)
                    dma_groups += 1
                if gi == 0:
                    emit_b_phase1(0, None)
                    for pr in range(4, SPL):
                        transpose_pair(xs16[pr][:], pr * 2)
                    prev = 0
                    continue
                while t_pairs < t_need[ck]:
                    emit_pair_T(t_pairs)
                    t_pairs += 1
                emit_b_phase1(gi, prev)
                prev = gi
            emit_p2_flush(prev)

    nc.compile()
    return nc


def _host_prep(W, e, bias, rows, cols):
    """Precompute fp16 device constants from the small parameter tensors."""
    W = np.asarray(W, np.float32)
    e = np.asarray(e, np.float32)
    bias = np.asarray(bias, np.float32)
    rows = np.asarray(rows, np.int64)
    cols = np.asarray(cols, np.int64)

    logits = np.full((HEADS, NLM, NLM), NEG, np.float64)
    logits[:, rows, cols] = e.astype(np.float64)
    m = logits.max(axis=-1, keepdims=True)
    p = np.exp(logits - m)
    A = p / p.sum(axis=-1, keepdims=True)            # [H, N, N]
    dg = np.einsum("hii->hi", A).copy()              # [H, N]
    A_off = A.copy()
    np.einsum("hii->hi", A_off)[:] = 0.0

    # Wall: [c, (s, h, d)] -> chunked [128, 2, 512]
    wr = W.transpose(2, 1, 0, 3).reshape(CIN, 2 * HEADS * HC)   # [c, shd]
    wall = np.ascontiguousarray(
        wr.reshape(2, P, 2 * HEADS * HC).transpose(1, 0, 2)
    ).astype(np.float16)

    # graph matrices: [j, (head, i)]; row 98 = all-ones bias row
    gm = np.zeros((P, HEADS, P), np.float32)
    for h in range(HEADS):
        gm[:NLM, h, :NLM] = A_off[h].T
        gm[NLM, h, :NLM] = 1.0
    gmat = np.ascontiguousarray(gm.reshape(P, HEADS * P)).astype(np.float16)

    dgvt = np.ascontiguousarray(dg.T).astype(np.float32)        # [98, 4]

    # bias row pattern for hgrp row 98: [s, (part, h, d)], part-1 = bias
    br = np.zeros((G, 2, HEADS * HC), np.float32)
    br[:, 1, :] = bias.reshape(HEADS * HC)
    brow = np.ascontiguousarray(br.reshape(1, G * HD)).astype(np.float16)

    ident = np.eye(P, dtype=np.float16)
    return {"wall": wall, "gmat": gmat, "dgvt": dgvt, "brow": brow,
            "ident": ident}


def kernel(x, W, e, bias, rows, cols):
    from concourse.bass_utils import run_bass_kernel_spmd

    if "nc" not in _CACHE:
        _CACHE["nc"] = _build_nc()
    nc = _CACHE["nc"]

    consts = _host_prep(W, e, bias, rows, cols)
    x16 = np.asarray(x, np.float32).reshape(B * NLM, CIN).astype(np.float16)

    in_maps = []
    for ci in range(NCORES):
        shard = np.ascontiguousarray(x16[ci * NS * NLM : (ci + 1) * NS * NLM])
        in_maps.append({"x16": shard, **consts})

    res = run_bass_kernel_spmd(
        nc,
        in_maps,
        core_ids=list(range(NCORES)),
        trace=bool(int(os.environ.get("KERNEL_TRACE", "0"))),
    )
    _CACHE["last_results"] = res

    out = np.concatenate(
        [r["out"].reshape(NS, NLM, HEADS * HC) for r in res.results], axis=0
    )
    return out


# revision 36
# speedup vs baseline: 1.2603x; 1.2603x over previous
"""MultiHeadSemGConv Trainium2 kernel.

Computes, for x:[B,N,CIN], W:[H,2,CIN,HC], e:[H,N*K], bias:[H,HC],
rows/cols:[N*K] (int32 edge list):

    h = einsum('bnc,hscd->shbnd', x, W)             # two projections per head
    A = softmax(scatter(e at (rows,cols), NEG))     # [H,N,N]
    out[h,b] = diag(A)*h0 + (A - diag)@h1 + bias    # -> [B,N,H*HC]

Strategy: pure data-parallel over batch across 8 NeuronCores.  The tiny
[H,98,98] adjacency softmax is precomputed on host; the heavy lifting
(x projection + graph mixing over 100MB of activations) runs on device:

  per core (128 samples, x pre-cast to fp16 host-side):
    - DMA x16 in flat 128-partition tiles (group 0: per-pair HWDGE loads
      on both queues for low head latency; later groups: bulk SWDGE)
    - PE transpose (matmul with identity) -> xT chunk tiles
      [c(2x128), 16*98+30 cols] fp16 in SBUF (30-col overlap keeps every
      per-sample phase-1 stationary at m=128)
    - phase 1, per sample b: h[128,512] = xT[:, 98b:98b+128].T @ Wall
      (2 accumulating fp16 matmuls, f32 PSUM), 2 samples per PSUM tile;
      rows [:98] copied per-sample-half on scalar+vector in parallel into
      one of two persistent h tiles whose row 98 holds the bias pattern
    - phase 2, per 8-sample group, per head: ONE matmul with the
      host-built A_off^T (contract k=99: 98 nodes + bias row), then a
      fused DVE op  out = dg (.) h0 + psum  adds the diagonal part.
      Phase 2 of group g is interleaved into phase 1 of group g+1.
    - DMA out f32
"""

import os
import sys

import numpy as np

try:
    import concourse.bass as bass  # noqa: F401
except Exception:  # pragma: no cover - fresh grading dir fallback
    for p in ("/opt/trn_rl_repo", "/root/.axon_site/_ro/trn_rl_repo"):
        if os.path.isdir(p) and p not in sys.path:
            sys.path.insert(0, p)
    import concourse.bass as bass  # noqa: F401

# ---------------------------------------------------------------- constants
NLM = 98          # landmarks (graph nodes)
HEADS = 4
CIN = 256
HC = 64
HD = 512          # h width = 2 (s) * 4 (heads) * 64 (d)
B = 1024
NCORES = 8
NS = B // NCORES  # samples per core = 128
P = 128
G = 8             # samples per output group
NGRP = NS // G    # 16 groups per core
OVL = 30          # overlap cols so every phase-1 lhsT can be m=128
NEG = -9e15

CHS = 16                    # samples per xT chunk
NCH = NS // CHS             # 8 chunks
CHW = CHS * NLM             # 1568 cols per chunk (+OVL)
NFT = NS * NLM // P         # 98 flat 128-row tiles
NPAIR = NFT // 2            # 49 transpose pairs
DGF = 14                    # flat tiles per input DMA group
NDG = NFT // DGF            # 7 DMA groups
SPL = 7                     # pairs in the split (head) group 0

_CACHE = {}


def _build_nc():
    import concourse.mybir as mybir
    import concourse.tile as tile
    from concourse import bacc

    f16 = mybir.dt.float16
    f32 = mybir.dt.float32
    MUL = mybir.AluOpType.mult
    ADD = mybir.AluOpType.add

    nc = bacc.Bacc(None, target_bir_lowering=False)

    x16 = nc.dram_tensor("x16", [NS * NLM, CIN], f16, kind="ExternalInput")
    wall = nc.dram_tensor("wall", [P, 2, HD], f16, kind="ExternalInput")
    gmat = nc.dram_tensor("gmat", [P, HEADS * P], f16, kind="ExternalInput")
    dgvt = nc.dram_tensor("dgvt", [NLM, HEADS], f32, kind="ExternalInput")
    brow = nc.dram_tensor("brow", [1, G * HD], f16, kind="ExternalInput")
    ident = nc.dram_tensor("ident", [P, P], f16, kind="ExternalInput")
    out = nc.dram_tensor("out", [NS * NLM, CIN], f32, kind="ExternalOutput")

    with tile.TileContext(nc) as tc:
        with (
            tc.tile_pool(name="const", bufs=1) as constp,
            tc.tile_pool(name="x0", bufs=1) as x0p,
            tc.tile_pool(name="xin", bufs=3) as xinp,
            tc.tile_pool(name="xt", bufs=1) as xtp,
            tc.tile_pool(name="hg", bufs=1) as hgp,
            tc.tile_pool(name="osb", bufs=4) as osbp,
            tc.tile_pool(name="ptr", bufs=2, space="PSUM") as ptrp,
            tc.tile_pool(name="phs", bufs=2, space="PSUM") as phsp,
            tc.tile_pool(name="pout", bufs=2, space="PSUM") as poutp,
        ):
            dgv_sb = constp.tile([NLM, HEADS], f32, tag="dgv")
            ident_sb = constp.tile([P, P], f16, tag="ident")
            nc.sync.dma_start(ident_sb[:], ident[:])

            xs16 = [
                x0p.tile([P, 2, CIN], f16, tag=f"xs{pr}", name=f"xs{pr}")
                for pr in range(SPL)
            ]
            wall_sb = constp.tile([P, 2, HD], f16, tag="wall")
            gm_sb = constp.tile([P, HEADS * P], f16, tag="gmat")

            def pair_dma(pr):
                # full-pair DMA; alternate the two HWDGE queues so
                # descriptor-gen parallelizes
                b0 = pr * 2 * P
                src = x16[b0 : b0 + 2 * P, :].rearrange("(t p) c -> p t c", p=P)
                eng = nc.sync if pr % 2 == 0 else nc.scalar
                eng.dma_start(xs16[pr][:], src)

            pair_dma(0)
            nc.scalar.dma_start(wall_sb[:], wall[:])
            for pr in range(1, 4):
                pair_dma(pr)

            hgt = [
                hgp.tile([P, G * HD], f16, tag=f"hg{i}", name=f"hg{i}")
                for i in range(2)
            ]

            xt = [
                xtp.tile([P, 2, CHW + OVL], f16, tag=f"xt{k}", name=f"xt{k}")
                for k in range(NCH)
            ]
            nc.vector.memset(xt[NCH - 1][:, :, CHW:], 0.0)

            def route_piece(g0, ptr, off, w):
                """Copy ptr[:, :, off:off+w] (global xT cols [g0,g0+w)) into
                the chunk tiles, including overlap duplication."""
                while w > 0:
                    k = g0 // CHW
                    lo = g0 - k * CHW
                    pw = min(w, CHW - lo)
                    nc.scalar.copy(
                        out=xt[k][:, :, lo : lo + pw],
                        in_=ptr[:, :, off : off + pw],
                    )
                    # overlap region of the previous chunk
                    if k > 0 and lo < OVL:
                        ow = min(pw, OVL - lo)
                        nc.scalar.copy(
                            out=xt[k - 1][:, :, CHW + lo : CHW + lo + ow],
                            in_=ptr[:, :, off : off + ow],
                        )
                    g0 += pw
                    off += pw
                    w -= pw

            def transpose_pair(xin_ap, gft):
                """Transpose 2 flat tiles (xin_ap: [P, 2, CIN] fp16) whose
                first global flat-tile index is gft."""
                ptr = ptrp.tile([P, 2, 2 * P], f32, tag="ptr")
                for a in range(2):
                    for cc in range(2):
                        nc.tensor.matmul(
                            ptr[:, cc, a * P : (a + 1) * P],
                            xin_ap[:, a, cc * P : (cc + 1) * P],
                            ident_sb[:],
                            start=True,
                            stop=True,
                        )
                route_piece(gft * P, ptr, 0, 2 * P)

            # ---- bulk input: SWDGE, DGF flat tiles a shot ---------------
            # The SWDGE descriptor-gens would otherwise all fire at t=0 and
            # flood the DMA rings ahead of the latency-critical head loads.
            # The Tile scheduler reorders by data deps, so gens 1-2 are
            # gated with a WAW dep: a tiny copy (itself gated on a head
            # pair load) into the DMA's own destination tile.  Later gens
            # are paced by the xin pool WAR (bufs=3).
            xin_tiles = {}

            def emit_a_dma(dg):
                xin = xinp.tile([P, DGF, CIN], f16, tag="xin")
                if dg == 1:
                    nc.gpsimd.tensor_copy(xin[0:1, 0, 0:4], xs16[1][0:1, 0, 0:4])
                elif dg == 2:
                    nc.gpsimd.tensor_copy(xin[0:1, 0, 0:4], xs16[3][0:1, 0, 0:4])
                base = dg * DGF * P
                nc.gpsimd.dma_start(
                    xin[:],
                    x16[base : base + DGF * P, :].rearrange(
                        "(t p) c -> p t c", p=P
                    ),
                )
                xin_tiles[dg] = xin

            def emit_pair_T(tp):
                """Transpose bulk pair tp (pairs SPL.. come from bulk)."""
                ft = tp * 2
                dg, pr = divmod(ft, DGF)
                xin = xin_tiles[dg]
                transpose_pair(xin[:, pr : pr + 2, :], ft)

            hg3s = [h[:].rearrange("p (s f) -> p s f", s=G) for h in hgt]

            def emit_p2_head(gi, hd, osb3):
                """Phase 2 for one head of group gi: one k=99 matmul
                (A_off^T + bias row), then fused  out = dg (.) h0 + psum."""
                hg3 = hg3s[gi % 2]
                pouts = poutp.tile([P, G * HC], f32, tag="pout")
                po3 = pouts[:].rearrange("p (s f) -> p s f", s=G)
                nc.tensor.matmul(
                    po3,
                    gm_sb[0:99, hd * P : (hd + 1) * P],
                    hg3[0:99, :, 256 + hd * HC : 256 + (hd + 1) * HC],
                    start=True,
                    stop=True,
                )
                nc.vector.scalar_tensor_tensor(
                    out=osb3[:, :, hd * HC : (hd + 1) * HC],
                    in0=hg3[0:98, :, hd * HC : (hd + 1) * HC],
                    scalar=dgv_sb[:, hd : hd + 1],
                    in1=po3[0:98],
                    op0=MUL,
                    op1=ADD,
                )

            def store(gi, osb3, s0=0, s1=G):
                ov = out[gi * G * NLM : (gi + 1) * G * NLM, :].rearrange(
                    "(s i) c -> i s c", s=G
                )
                nc.sync.dma_start(ov[:, s0:s1], osb3[:, s0:s1])

            osb_t = {}

            def emit_b_phase1(gi, prev):
                """Phase 1 for G samples of gi; phase 2 of group `prev`
                interleaved between the pairs."""
                hgrp = hgt[gi % 2]
                if prev is not None:
                    osb = osbp.tile([NLM, G * 256], f32, tag="osb")
                    osb3 = osb[:].rearrange("p (s c) -> p s c", s=G)
                    osb_t[prev] = osb3
                ck = gi // 2
                for pi in range(G // 2):
                    hps = phsp.tile([P, 2, HD], f32, tag="hps")
                    for a in range(2):
                        b = gi * G + pi * 2 + a
                        lb = b - ck * CHS
                        for cc in range(2):
                            nc.tensor.matmul(
                                hps[:, a, :],
                                xt[ck][:, cc, NLM * lb : NLM * lb + P],
                                wall_sb[:, cc, :],
                                start=(cc == 0),
                                stop=(cc == 1),
                            )
                    dst = hgrp[0:98, pi * 2 * HD : (pi + 1) * 2 * HD].rearrange(
                        "p (a f) -> p a f", a=2
                    )
                    if pi == 0:
                        nc.vector.tensor_copy(dst, hps[0:98])
                    else:
                        nc.scalar.copy(out=dst, in_=hps[0:98])
                    if prev is not None:
                        if pi < 3:
                            emit_p2_head(prev, pi, osb_t[prev])
                        else:
                            emit_p2_head(prev, 3, osb_t[prev])
                            store(prev, osb_t[prev])

            def emit_p2_flush(gi):
                """Phase 2 for the final group, split in half-groups with
                split stores for a short kernel tail."""
                osb = osbp.tile([NLM, G * 256], f32, tag="osb")
                osb3 = osb[:].rearrange("p (s c) -> p s c", s=G)
                hg3 = hg3s[gi % 2]
                for half in range(2):
                    s0, s1 = half * 4, half * 4 + 4
                    for hd in range(HEADS):
                        pouts = poutp.tile([P, G * HC], f32, tag="pout")
                        po3 = pouts[:].rearrange("p (s f) -> p s f", s=G)
                        nc.tensor.matmul(
                            po3[:, s0:s1, :],
                            gm_sb[0:99, hd * P : (hd + 1) * P],
                            hg3[0:99, s0:s1, 256 + hd * HC : 256 + (hd + 1) * HC],
                            start=True,
                            stop=True,
                        )
                        nc.vector.scalar_tensor_tensor(
                            out=osb3[:, s0:s1, hd * HC : (hd + 1) * HC],
                            in0=hg3[0:98, s0:s1, hd * HC : (hd + 1) * HC],
                            scalar=dgv_sb[:, hd : hd + 1],
                            in1=po3[0:98, s0:s1],
                            op0=MUL,
                            op1=ADD,
                        )
                    if half == 0:
                        store(gi, osb3, 0, 4)
                    else:
                        store(gi, osb3, 4, 6)
                        store(gi, osb3, 6, 8)

            # ---- main emission ------------------------------------------
            t_need = [
                -(-(CHW * (k + 1) + OVL) // (2 * P)) for k in range(NCH)
            ]
            t_need[NCH - 1] = NPAIR
            dg_need = [min(-(-(2 * t_need[k]) // DGF), NDG) for k in range(NCH)]

            # group-0 head: 4 pairs -> b0 -> 3 pairs
            transpose_pair(xs16[0][:], 0)
            transpose_pair(xs16[1][:], 2)
            for pr in range(4, SPL):
                pair_dma(pr)
            nc.sync.dma_start(gm_sb[:], gmat[:])
            nc.scalar.dma_start(dgv_sb[:], dgvt[:])
            nc.sync.dma_start(hgt[0][98:99, :], brow[:])
            nc.sync.dma_start(hgt[1][98:99, :], brow[:])
            transpose_pair(xs16[2][:], 4)
            transpose_pair(xs16[3][:], 6)
            dma_groups = 1  # group 0 done via split path
            t_pairs = SPL
            prev = None
            for gi in range(NGRP):
                ck = gi // 2
                while dma_groups < dg_need[min((gi + 2) // 2, NCH - 1)]:
                    emit_a_dma(dma_groups)
                    dma_groups += 1
                if gi == 0:
                    emit_b_phase1(0, None)
                    for pr in range(4, SPL):
                        transpose_pair(xs16[pr][:], pr * 2)
                    prev = 0
                    continue
                while t_pairs < t_need[ck]:
                    emit_pair_T(t_pairs)
                    t_pairs += 1
                emit_b_phase1(gi, prev)
                prev = gi
            emit_p2_flush(prev)

    nc.compile()
    return nc


def _host_prep(W, e, bias, rows, cols):
    """Precompute fp16 device constants from the small parameter tensors."""
    W = np.asarray(W, np.float32)
    e = np.asarray(e, np.float32)
    bias = np.asarray(bias, np.float32)
    rows = np.asarray(rows, np.int64)
    cols = np.asarray(cols, np.int64)

    logits = np.full((HEADS, NLM, NLM), NEG, np.float64)
    logits[:, rows, cols] = e.astype(np.float64)
    m = logits.max(axis=-1, keepdims=True)
    p = np.exp(logits - m)
    A = p / p.sum(axis=-1, keepdims=True)            # [H, N, N]
    dg = np.einsum("hii->hi", A).copy()              # [H, N]
    A_off = A.copy()
    np.einsum("hii->hi", A_off)[:] = 0.0

    # Wall: [c, (s, h, d)] -> chunked [128, 2, 512]
    wr = W.transpose(2, 1, 0, 3).reshape(CIN, 2 * HEADS * HC)   # [c, shd]
    wall = np.ascontiguousarray(
        wr.reshape(2, P, 2 * HEADS * HC).transpose(1, 0, 2)
    ).astype(np.float16)

    # graph matrices: [j, (head, i)]; row 98 = all-ones bias row
    gm = np.zeros((P, HEADS, P), np.float32)
    for h in range(HEADS):
        gm[:NLM, h, :NLM] = A_off[h].T
        gm[NLM, h, :NLM] = 1.0
    gmat = np.ascontiguousarray(gm.reshape(P, HEADS * P)).astype(np.float16)

    dgvt = np.ascontiguousarray(dg.T).astype(np.float32)        # [98, 4]

    # bias row pattern for hgrp row 98: [s, (part, h, d)], part-1 = bias
    br = np.zeros((G, 2, HEADS * HC), np.float32)
    br[:, 1, :] = bias.reshape(HEADS * HC)
    brow = np.ascontiguousarray(br.reshape(1, G * HD)).astype(np.float16)

    ident = np.eye(P, dtype=np.float16)
    return {"wall": wall, "gmat": gmat, "dgvt": dgvt, "brow": brow,
            "ident": ident}


def kernel(x, W, e, bias, rows, cols):
    from concourse.bass_utils import run_bass_kernel_spmd

    if "nc" not in _CACHE:
        _CACHE["nc"] = _build_nc()
    nc = _CACHE["nc"]

    consts = _host_prep(W, e, bias, rows, cols)
    x16 = np.asarray(x, np.float32).reshape(B * NLM, CIN).astype(np.float16)

    in_maps = []
    for ci in range(NCORES):
        shard = np.ascontiguousarray(x16[ci * NS * NLM : (ci + 1) * NS * NLM])
        in_maps.append({"x16": shard, **consts})

    res = run_bass_kernel_spmd(
        nc,
        in_maps,
        core_ids=list(range(NCORES)),
        trace=bool(int(os.environ.get("KERNEL_TRACE", "0"))),
    )
    _CACHE["last_results"] = res

    out = np.concatenate(
        [r["out"].reshape(NS, NLM, HEADS * HC) for r in res.results], axis=0
    )
    return out
